# revision 38
# baseline (speedup 1.0000x reference)
"""Causal self-attention (B=2, N=2048, D=1024, H=16) on 8 trn2 NeuronCores.

Sharding: data-parallel over batch (2) x tensor-parallel over heads (4 head
groups of 4 heads) = 8 cores. Each core computes QKV projection for its 4
heads, causal attention, and its partial of the output projection (contraction
over its heads' dims). The host sums the 4 partials per batch element and adds
the constant term (out bias + v-bias routed through W_out, exact because
softmax rows sum to 1).

Per-core kernel, all matmul operands in bf16 (same 1 cycle/row PE stream
rate as float32r, but FWL-fast weight loads and half the DMA bytes; fp8 was
numerically rejected — simulated rel err 2.4e-2..5.4e-2 vs the 2e-2 gate):
  - the whole x panel (4 MB bf16) is resident in SBUF; startup DMA pairs
    wqk_f with the chunk-0 x quarter per f on a rotating queue so the first
    QKV chains are paced by per-f arrivals (~0.6 us/trigger sequencer cost,
    only sync/scalar/gpsimd host DMA queues);
  - q,k kept transposed [head_dim, tokens]; v kept natural [tokens, head_dim]
    with a static ones column per head (gpsimd memset) so the p@v matmul also
    produces the softmax denominators;
  - scores computed transposed st[j,i] = k_j . q_i for 128-row j-tiles and
    512-col i-chunks; two heads packed side by side into one [128,1024] PSUM
    tile (row-disjoint K=64 pairs overlap in the PE);
  - exp on ACT with scale=1/8 straight out of PSUM (no max subtraction; the
    score range for these inputs is a few units). Causal masking only on the
    diagonal blocks via gpsimd affine_select; diagonal j-tiles are processed
    early in each chunk (order [0, njt-1..1]) so their serialized
    exp->select->pv chains hide under filler matmuls — jt=0 stays first
    because its full-width pv initializes all PSUM columns;
  - softmax normalization deferred past p@v using the ones-column sums;
  - emission interleaves next-chunk QKV and prev-chunk out-projection matmuls
    into the attention block stream so the PE never idles long enough for the
    HAM clock gate to re-throttle. QKV accumulation chains are emitted as
    interleaved pairs (alternating PSUM banks) so weight loads pipeline;
  - endgame: the last chunk runs head-pair 1 first, its tails drain during
    pair 0's blocks; pair 0's normalization reads PSUM directly and is
    interleaved with the final out-projection in two 4-chain waves (aoT[1]
    side first — runnable immediately — staggered via a 1-element touch so
    the PE clock stays warm; aoT[0] closers follow the muls), with copies on
    the by-then-idle scalar engine.
"""

import sys
from collections import deque

if '/opt/trn_rl_repo' not in sys.path:
    sys.path.insert(0, '/opt/trn_rl_repo')

import numpy as np
import ml_dtypes

import concourse.bacc as bacc
import concourse.mybir as mybir
import concourse.tile as tile
from concourse.bass_utils import run_bass_kernel_spmd

F32 = mybir.dt.float32
BF16 = mybir.dt.bfloat16
EXP = mybir.ActivationFunctionType.Exp
MULT = mybir.AluOpType.mult
ADD = mybir.AluOpType.add
IS_GE = mybir.AluOpType.is_ge

NP_BF16 = ml_dtypes.bfloat16

B, N, D, H = 2, 2048, 1024, 16
HD = D // H          # 64
HPC = 4              # heads per core
NCORES = 8
NT = N // 512        # 4 token chunks of 512
NJ = N // 128        # 16 key tiles of 128
SCALE = HD ** (-0.5)

USE_RECIP_FAST = True


def _emit(ctx, nc, tc, dram):
    xT, wqk, wv, wo, bqk, y = (
        dram['xT'], dram['wqk'], dram['wv'], dram['wo'], dram['bqk'],
        dram['y'])

    cp = ctx.enter_context(tc.tile_pool(name="const", bufs=1))
    pp = ctx.enter_context(tc.tile_pool(name="pexp", bufs=6))
    sm = ctx.enter_context(tc.tile_pool(name="small", bufs=4))
    psc = ctx.enter_context(tc.tile_pool(name="psc", bufs=2, space="PSUM"))
    pao = ctx.enter_context(tc.tile_pool(name="pao", bufs=2, space="PSUM"))
    pms = ctx.enter_context(tc.tile_pool(name="pms", bufs=2, space="PSUM"))

    # ---- persistent tiles -------------------------------------------------
    xf = [cp.tile([128, N], BF16, tag=f"xf{f}", name=f"xf{f}")
          for f in range(8)]
    wqk_t = [cp.tile([128, 512], BF16, tag=f"wqk{f}", name=f"wqk{f}")
             for f in range(8)]
    wv_t = [cp.tile([128, 256], BF16, tag=f"wv{f}", name=f"wv{f}")
            for f in range(8)]
    wo_t = [cp.tile([128, 1024], BF16, tag=f"wo{k}", name=f"wo{k}")
            for k in range(2)]
    bqk_t = cp.tile([128, 4], F32, tag="bqk", name="bqk")
    # q,k transposed: 4 tiles [128 dims, 2048 tokens]; rt 0,1 = q; rt 2,3 = k
    qkT = [cp.tile([128, N], BF16, tag=f"qkT{r}", name=f"qkT{r}")
           for r in range(4)]
    # v natural per j-tile with ones columns: [v_h0|1|v_h1|1|v_h2|1|v_h3|1]
    v_t = [cp.tile([128, 65 * HPC], BF16, tag=f"v{j}", name=f"v{j}")
           for j in range(NJ)]
    # normalized attention output, transposed [head dims, tokens]
    aoT = [cp.tile([128, N], BF16, tag=f"aoT{k}", name=f"aoT{k}")
           for k in range(2)]
    # ---- startup DMA ------------------------------------------------------
    # Only sync/scalar(Act)/gpsimd host DMA queues, and each trigger costs
    # ~0.6 us of that sequencer. scalar must be clear before the first EXP
    # (~8 us) and gpsimd before the first affine_select, so: x half-tiles
    # alternate sync/scalar (h=0 covers chunks 0-1 and lands first), wqk on
    # gpsimd (first chains pace on it), wv/wo on sync behind the h0 tiles.
    nc.scalar.dma_start(out=bqk_t[:], in_=bqk[:, :])
    # pair wqk_f with x_f(h0) on a rotating queue so the first qkv chain is
    # paced by per-f arrivals instead of waiting for a whole queue to drain
    qrot = (nc.sync, nc.scalar, nc.gpsimd)
    for f in range(8):
        q = qrot[f % 3]
        q.dma_start(out=wqk_t[f][:], in_=wqk[f * 128:(f + 1) * 128, :])
        # chunk 0 only needs the first 512 columns; quarter loads keep the
        # first qkv chains fed at the PE's pace
        q.dma_start(out=xf[f][:, 0:512],
                    in_=xT[f * 128:(f + 1) * 128, 0:512])
    for f in range(8):
        nc.sync.dma_start(out=xf[f][:, 512:1024],
                          in_=xT[f * 128:(f + 1) * 128, 512:1024])
    for f in range(8):
        q = nc.gpsimd if f < 4 else nc.scalar
        q.dma_start(out=wv_t[f][:], in_=wv[f * 128:(f + 1) * 128, :])
    for k in range(2):
        nc.sync.dma_start(out=wo_t[k][:], in_=wo[k * 128:(k + 1) * 128, :])
    for f in range(8):
        nc.sync.dma_start(out=xf[f][:, 1024:2048],
                          in_=xT[f * 128:(f + 1) * 128, 1024:2048])

    # ---- task generators (each yielded thunk emits ~one PE instruction) ---
    def qk_tasks(c):
        # q,k transposed r-tiles, emitted as interleaved pairs so consecutive
        # matmuls hit alternating PSUM banks (weight loads pipeline).
        for pa in range(2):
            sts = [{}, {}]
            def _mk(rt, f, st):
                def _t():
                    if f == 0:
                        st['ps'] = pms.tile([128, 512], F32, tag="ms",
                                            name=f"qk{c}_{rt}")
                    nc.tensor.matmul(st['ps'][:],
                                     wqk_t[f][:, rt * 128:(rt + 1) * 128],
                                     xf[f][:, c * 512:(c + 1) * 512],
                                     start=(f == 0), stop=(f == 7))
                    if f == 7:
                        nc.vector.tensor_scalar_add(
                            qkT[rt][:, c * 512:(c + 1) * 512], st['ps'][:],
                            bqk_t[:, rt:rt + 1])
                return _t
            for f in range(8):
                for i in range(2):
                    yield _mk(2 * pa + i, f, sts[i])

    def v_tasks(c):
        # v natural t-tiles (pairs likewise)
        for pa in range(2):
            sts = [{}, {}]
            def _mkv(tt, jt, f, st):
                def _t():
                    if f == 0:
                        st['ps'] = pms.tile([128, 256], F32, tag="ms",
                                            name=f"v{c}_{tt}")
                    nc.tensor.matmul(st['ps'][:],
                                     xf[f][:, c * 512 + tt * 128:
                                           c * 512 + (tt + 1) * 128],
                                     wv_t[f][:], start=(f == 0), stop=(f == 7))
                    if f == 7:
                        ps = st['ps']
                        v3 = v_t[jt][:].rearrange("p (g e) -> p g e", e=65)
                        # static ones columns, written on gpsimd in parallel
                        # with the vector copy of the v values
                        nc.gpsimd.memset(v3[:, :, 64:65], 1.0)
                        nc.vector.tensor_copy(
                            out=v3[:, :, 0:64],
                            in_=ps[:].rearrange("p (g e) -> p g e", e=64))
                return _t
            for f in range(8):
                for i in range(2):
                    tt = 2 * pa + i
                    yield _mkv(tt, 4 * c + tt, f, sts[i])

    def qkv_tasks(c):
        yield from qk_tasks(c)
        yield from v_tasks(c)

    def yproj_tasks(c):
        for tt in range(4):
            t0 = c * 512 + tt * 128
            st = {}
            for ec in range(2):
                def _mk(t0, ec, k, st, qi):
                    def _t():
                        if ec == 0 and k == 0:
                            st['y'] = sm.tile([128, 1024], BF16, tag="y",
                                              name=f"y{t0}", bufs=3)
                        if k == 0:
                            st['ps'] = pms.tile([128, 512], F32, tag="ms",
                                                name=f"yp{t0}_{ec}")
                        nc.tensor.matmul(
                            st['ps'][:], aoT[k][:, t0:t0 + 128],
                            wo_t[k][:, ec * 512:(ec + 1) * 512],
                            start=(k == 0), stop=(k == 1))
                        if k == 1:
                            nc.vector.tensor_copy(
                                out=st['y'][:, ec * 512:(ec + 1) * 512],
                                in_=st['ps'][:])
                            if ec == 1:
                                # one full-row DMA per token tile: 2 KB
                                # descriptor lines, half the trigger count
                                dq = (nc.sync, nc.gpsimd)[qi]
                                dq.dma_start(out=y[t0:t0 + 128, :],
                                             in_=st['y'][:, :])
                    return _t
                qi = tt % 2
                for k in range(2):
                    yield _mk(t0, ec, k, st, qi)

    def yproj_last_wave(wave, st):
        # endgame out-projection wave of 4 chains: the aoT[1]-side matmuls
        # are runnable the moment the last pv matmul retires (pr=1 finished
        # mid-chunk) — real work that keeps the PE clock warm through the
        # norm tails; the aoT[0]-side closers run after the tails' muls.
        # psc's banks are idle by now, so waves borrow them for 4 chains.
        c = NT - 1
        for tt in wave:
            t0 = c * 512 + tt * 128
            yt = sm.tile([128, 1024], BF16, tag="y", name=f"y{t0}", bufs=3)
            for ec in range(2):
                pool, tag = (pms, "ms") if tt % 2 == 0 else (psc, "sc")
                ps = pool.tile([128, 512], F32, tag=tag, name=f"yl{t0}_{ec}")
                nc.tensor.matmul(
                    ps[:], aoT[1][:, t0:t0 + 128],
                    wo_t[1][:, ec * 512:(ec + 1) * 512],
                    start=True, stop=False)
                st[(tt, ec)] = (ps, yt)

    def yproj_last_close(wave, st):
        c = NT - 1
        for tt in wave:
            t0 = c * 512 + tt * 128
            for ec in range(2):
                ps, yt = st[(tt, ec)]
                nc.tensor.matmul(
                    ps[:], aoT[0][:, t0:t0 + 128],
                    wo_t[0][:, ec * 512:(ec + 1) * 512],
                    start=False, stop=True)
                nc.scalar.activation(
                    yt[:, ec * 512:(ec + 1) * 512], ps[:],
                    mybir.ActivationFunctionType.Copy)
                if ec == 1:
                    dq = (nc.sync, nc.gpsimd)[tt % 2]
                    dq.dma_start(out=y[t0:t0 + 128, :], in_=yt[:, :])

    def emit_pv(pr, jt, p_entry, ao_e, ao_o, first, last):
        p, i0 = p_entry
        for g, ao in ((2 * pr, ao_e), (2 * pr + 1, ao_o)):
            nc.tensor.matmul(
                ao[:, i0:512], v_t[jt][:, g * 65:g * 65 + 65],
                p[:, (g % 2) * 512 + i0:(g % 2) * 512 + 512],
                start=first, stop=last)

    # ---- main schedule ----------------------------------------------------
    rscratch_t = nc.dram_tensor("rscratch", [16, 1, 512], F32)
    rscratch = [rscratch_t.ap()[i] for i in range(16)]
    fill = deque()
    last_aos = []
    # chunk-1 qk chains run between chunk-0's qk and v parts: their x data
    # lands right behind chunk 0's, and they cover the window where the wv
    # weights are still in flight
    for t in qk_tasks(0):
        t()
    for t in qk_tasks(1):
        t()
    for t in v_tasks(0):
        t()

    for c in range(NT):
        if c == 0:
            fill.extend(v_tasks(1))
        elif c + 1 < NT:
            fill.extend(qkv_tasks(c + 1))
        if c >= 1:
            fill.extend(yproj_tasks(c - 1))

        njt = 4 * c + 4
        nblocks = 2 * njt
        blk = 0

        # keep a hard reserve mid-kernel so attention blocks always have PE
        # cover; on the last chunk drain everything early (finish ~8 blocks
        # before the end) so the endgame's PSUM pools and the vector queue
        # are clear when the final out-projection waves want them
        reserve = 12 if c < NT - 1 else 0
        lead = 8 if c == NT - 1 else 0

        def drain_fillers(blocks_left):
            avail = max(0, len(fill) - reserve)
            want = -(-avail // max(blocks_left - lead, 1))  # ceil
            for _ in range(min(want, avail)):
                fill.popleft()()

        # last chunk: pr=1 first, so its norm tails drain during pr=0's
        # attention blocks instead of serializing before the final out-proj
        prs = (0, 1) if c < NT - 1 else (1, 0)
        for pr in prs:
            qt, kt = qkT[pr], qkT[2 + pr]
            ao_e = pao.tile([65, 512], F32, tag="ao", name=f"aoe{c}_{pr}")
            ao_o = pao.tile([65, 512], F32, tag="ao", name=f"aoo{c}_{pr}")
            # Process the diagonal (masked) j-tiles early — their serialized
            # exp -> affine_select -> pv chains then overlap with filler work
            # instead of pacing the end of the chunk. jt=0 must stay first:
            # its full-width pv matmul is the one that initializes (start=
            # True) all 512 PSUM columns of the ao accumulators.
            order = [0] + list(range(njt - 1, 0, -1))
            plist = []
            for bi, jt in enumerate(order):
                # diagonal blocks: columns i < i0 are fully masked, skip them
                d = jt - 4 * c
                i0 = 128 * d if d >= 1 else 0
                w = 512 - i0
                sc = psc.tile([128, 1024], F32, tag="sc",
                              name=f"sc{c}_{pr}_{jt}")
                nc.tensor.matmul(sc[:, i0:512],
                                 kt[0:64, jt * 128:(jt + 1) * 128],
                                 qt[0:64, c * 512 + i0:(c + 1) * 512],
                                 start=True, stop=True)
                nc.tensor.matmul(sc[:, 512 + i0:1024],
                                 kt[64:128, jt * 128:(jt + 1) * 128],
                                 qt[64:128, c * 512 + i0:(c + 1) * 512],
                                 start=True, stop=True)
                p = pp.tile([128, 1024], BF16, tag="p", name=f"p{c}_{pr}_{jt}")
                p3 = p[:].rearrange("p (h i) -> p h i", i=512)[:, :, i0:512]
                sc3 = sc[:].rearrange("p (h i) -> p h i", i=512)[:, :, i0:512]
                nc.scalar.activation(p3, sc3, EXP, scale=SCALE)
                if d >= 0:
                    nc.gpsimd.affine_select(
                        out=p3, in_=p3, compare_op=IS_GE, fill=0.0,
                        base=0, channel_multiplier=-1,
                        pattern=[[0, 2], [1, w]])
                plist.append((jt, p, i0))
                if bi >= 1:
                    pjt, pp_, pi0 = plist[bi - 1]
                    emit_pv(pr, pjt, (pp_, pi0), ao_e, ao_o,
                            first=(bi == 1), last=False)
                blk += 1
                drain_fillers(nblocks - blk)
            ljt, lp, li0 = plist[-1]
            emit_pv(pr, ljt, (lp, li0), ao_e, ao_o,
                    first=(njt == 1), last=True)

            # normalization: copy PSUM out fast (frees the ao banks); the
            # reciprocal+broadcast+mul tail is deferred into the next chunk's
            # filler stream so it never gates this pipeline. The very last
            # head-pair has no successor to free banks for; its tails are
            # emitted inline in the endgame below, reading PSUM directly.
            last_pr = (c == NT - 1 and pr == prs[-1])
            for g, ao in ((2 * pr, ao_e), (2 * pr + 1, ao_o)):
                if last_pr:
                    last_aos.append(ao)
                    continue
                src = sm.tile([65, 512], F32, tag="aosb",
                              name=f"aosb{c}_{g}")
                nc.vector.tensor_copy(out=src[:], in_=ao[:])

                def _norm_tail(c=c, pr=pr, g=g, src=src):
                    r = sm.tile([1, 512], F32, tag="r", name=f"r{c}_{g}",
                                bufs=2)
                    # custom-DVE ops don't handle nonzero partition offsets;
                    # stage the sums row at partition 0 first.
                    s_row = sm.tile([1, 512], F32, tag="srow",
                                    name=f"srow{c}_{g}", bufs=2)
                    nc.vector.tensor_copy(out=s_row[:], in_=src[64:65, :])
                    nc.vector.reciprocal_approx_fast(out=r[:], in_=s_row[:])
                    # broadcast via a DRAM bounce (keeps gpsimd free for the
                    # causal-mask selects; an SBUF-source broadcast AP is not
                    # expressible).
                    rb = sm.tile([64, 512], F32, tag="rb", name=f"rb{c}_{g}",
                                 bufs=2)
                    rd = rscratch[4 * c + g]
                    nc.sync.dma_start(out=rd, in_=r[:])
                    nc.sync.dma_start(out=rb[:],
                                      in_=rd.to_broadcast([64, 512]))
                    nc.vector.tensor_mul(
                        aoT[pr][(g % 2) * 64:(g % 2) * 64 + 64,
                                c * 512:(c + 1) * 512],
                        src[0:64, :], rb[:])
                fill.append(_norm_tail)

    while fill:
        fill.popleft()()

    # ---- endgame: final head-pair normalization interleaved with the ------
    # out-projection waves. The pr=1 side (aoT[1]) finished mid-chunk, so
    # its matmuls fire the moment the last pv retires; a 1-element "touch"
    # on the tt=1 token slice delays the second pair to mid-tail, keeping
    # the PE clock warm until the aoT[0]-side closers become runnable.
    c3 = NT - 1
    cols = slice(c3 * 512, (c3 + 1) * 512)
    ao0, ao1 = last_aos
    st1 = {}
    yproj_last_wave((0,), st1)
    sr0 = sm.tile([1, 512], F32, tag="srow", name="srow_l0", bufs=2)
    nc.scalar.activation(sr0[:], ao0[64:65, :],
                         mybir.ActivationFunctionType.Copy)
    sr1 = sm.tile([1, 512], F32, tag="srow", name="srow_l1", bufs=2)
    nc.vector.tensor_copy(out=sr1[:], in_=ao1[64:65, :])
    r0 = sm.tile([1, 512], F32, tag="r", name="r_l0", bufs=2)
    nc.vector.reciprocal_approx_fast(out=r0[:], in_=sr0[:])
    t1c = c3 * 512 + 128
    nc.vector.tensor_copy(out=aoT[1][96:97, t1c:t1c + 1],
                          in_=aoT[1][96:97, t1c:t1c + 1])
    yproj_last_wave((1,), st1)
    r1 = sm.tile([1, 512], F32, tag="r", name="r_l1", bufs=2)
    nc.vector.reciprocal_approx_fast(out=r1[:], in_=sr1[:])
    rb0 = sm.tile([64, 512], F32, tag="rb", name="rb_l0", bufs=2)
    nc.gpsimd.partition_broadcast(rb0[:], r0[:])
    rb1 = sm.tile([64, 512], F32, tag="rb", name="rb_l1", bufs=2)
    nc.gpsimd.partition_broadcast(rb1[:], r1[:])
    nc.vector.tensor_mul(aoT[0][0:64, cols], ao0[0:64, :], rb0[:])
    nc.vector.tensor_mul(aoT[0][64:128, cols], ao1[0:64, :], rb1[:])
    yproj_last_close((0, 1), st1)
    st2 = {}
    yproj_last_wave((2, 3), st2)
    yproj_last_close((2, 3), st2)


_CACHE = {}


def _build():
    if 'nc' in _CACHE:
        return _CACHE['nc']
    nc = bacc.Bacc("TRN2", target_bir_lowering=False, debug=False)
    dram = {
        'xT': nc.dram_tensor("xT", [D, N], BF16, kind="ExternalInput").ap(),
        'wqk': nc.dram_tensor("wqk", [D, 512], BF16, kind="ExternalInput").ap(),
        'wv': nc.dram_tensor("wv", [D, 256], BF16, kind="ExternalInput").ap(),
        'wo': nc.dram_tensor("wo", [256, D], BF16, kind="ExternalInput").ap(),
        'bqk': nc.dram_tensor("bqk", [128, 4], F32, kind="ExternalInput").ap(),
        'y': nc.dram_tensor("y", [N, D], BF16, kind="ExternalOutput").ap(),
    }
    from contextlib import ExitStack
    with tile.TileContext(nc) as tc, ExitStack() as ctx:
        _emit(ctx, nc, tc, dram)
    nc.compile()
    _CACHE['nc'] = nc
    return nc


def _prep_core_inputs(x, W_qkv, b_qkv, W_out, core):
    b = core // 4
    h0 = HPC * (core % 4)
    r0 = HD * h0
    q_rows = W_qkv[r0:r0 + 256]
    k_rows = W_qkv[D + r0:D + r0 + 256]
    v_rows = W_qkv[2 * D + r0:2 * D + r0 + 256]
    bqk_cat = np.concatenate(
        [b_qkv[r0:r0 + 256], b_qkv[D + r0:D + r0 + 256]])
    return {
        'xT': np.ascontiguousarray(x[b].T).astype(NP_BF16),
        'wqk': np.ascontiguousarray(
            np.concatenate([q_rows, k_rows], 0).T).astype(NP_BF16),
        'wv': np.ascontiguousarray(v_rows.T).astype(NP_BF16),
        'wo': np.ascontiguousarray(W_out[:, r0:r0 + 256].T).astype(NP_BF16),
        'bqk': np.ascontiguousarray(bqk_cat.reshape(4, 128).T),
    }


def kernel(x, W_qkv, b_qkv, W_out, b_out, _trace=False, _tmpdir=None):
    x = np.asarray(x, dtype=np.float32)
    W_qkv = np.asarray(W_qkv, dtype=np.float32)
    b_qkv = np.asarray(b_qkv, dtype=np.float32)
    W_out = np.asarray(W_out, dtype=np.float32)
    b_out = np.asarray(b_out, dtype=np.float32)

    in_maps = [_prep_core_inputs(x, W_qkv, b_qkv, W_out, c)
               for c in range(NCORES)]
    nc = _build()
    res = run_bass_kernel_spmd(nc, in_maps, list(range(NCORES)),
                               trace=_trace, tmpdir=_tmpdir)

    # v-bias contribution (softmax rows sum to 1) + output bias, as one
    # constant vector added on the host.
    bv = b_qkv[2 * D:3 * D]
    const = (b_out.astype(np.float64)
             + W_out.astype(np.float64) @ bv.astype(np.float64))
    out = np.empty((B, N, D), dtype=np.float32)
    for b in range(B):
        acc = np.zeros((N, D), dtype=np.float64)
        for g in range(4):
            acc += res.results[4 * b + g]['y'].astype(np.float64)
        out[b] = (acc + const).astype(np.float32)
    if _trace:
        kernel.last_exec_time_ns = res.exec_time_ns
        kernel.last_trace = (res.instructions_and_trace[1]
                             if res.instructions_and_trace else None)
    return out
